# revision 9
# baseline (speedup 1.0000x reference)
"""Batch Child-Sum TreeLSTM cell on 8 Trainium2 NeuronCores.

Strategy (data-parallel over nodes):
  - Shard the N nodes (and their contiguous 3-child edge segments) evenly
    across the 8 cores; replicate the small weight matrices.
  - On the host, lay activations out feature-major ("transposed", features on
    SBUF partitions) so every matmul contraction is along partitions and all
    segment sums are cheap strided free-axis ops.
  - All matmuls run as float32r (full PE rate at moving free-dim >= 256).
  - The child-sum (h_tilde) is folded into the big matmul as three extra
    accumulating matmuls over stride-3 views of child_h^T.
  - The per-edge forget-gate preactivation W_f x_j + U_f h_k is accumulated in
    a single PSUM tile (broadcast moving AP for the x term), and ScalarE
    evacuates it as sigmoid(psum + b_f) in one op.
"""

from contextlib import ExitStack

import numpy as np

import concourse.bass as bass
import concourse.bacc as bacc
import concourse.tile as tile
from concourse import mybir
from concourse.bass_utils import run_bass_kernel_spmd

F32 = mybir.dt.float32
F32R = mybir.dt.float32r

N_CORES = 8

# Tiling (in nodes). MACRO: DMA granularity for x / outputs. SUB: matmul
# moving free dim / PSUM tile width. FGRP: node group per forget-gate PSUM
# tile (FGRP * cpn must be <= 512 to fit one PSUM bank / one fp32 matmul,
# and even — walrus's fp32r dst-mem-pattern check rejects odd free dims).
MACRO = 2500
SUB = 500
FGRP = 170


def _ceil_div(a, b):
    return (a + b - 1) // b


def _chunks(total, step):
    out = []
    off = 0
    while off < total:
        out.append((off, min(step, total - off)))
        off += step
    return out


def build_program(npc, in_dim, hid, cpn, engines=None):
    """Bass program for one core's shard: npc nodes, npc*cpn edges."""
    assert in_dim % 128 == 0 and hid == 128
    kx = in_dim // 128  # k-chunks of the input features
    epc = npc * cpn

    eng = {"fjc": "gpsimd", "fc": "vector", "gates": "vector"}
    if engines:
        eng.update(engines)

    nc = bacc.Bacc("TRN2", target_bir_lowering=False, debug=False)

    xT = nc.dram_tensor("xT", [in_dim, npc], F32R, kind="ExternalInput").ap()
    chT = nc.dram_tensor("chT", [hid, epc], F32R, kind="ExternalInput").ap()
    ccT = nc.dram_tensor("ccT", [hid, epc], F32, kind="ExternalInput").ap()
    # W_combined [in+hid, 3*hid] row-major, sliced into [128,128] chunks.
    Wc = nc.dram_tensor("Wc", [in_dim + hid, 3 * hid], F32R, kind="ExternalInput").ap()
    Wf = nc.dram_tensor("Wf", [in_dim, hid], F32R, kind="ExternalInput").ap()
    Uf = nc.dram_tensor("Uf", [hid, hid], F32R, kind="ExternalInput").ap()
    bc = nc.dram_tensor("bc", [hid, 3], F32, kind="ExternalInput").ap()  # b_combined.reshape(3,128).T
    bf = nc.dram_tensor("bf", [hid, 1], F32, kind="ExternalInput").ap()

    cT = nc.dram_tensor("cT", [hid, npc], F32, kind="ExternalOutput").ap()
    hT = nc.dram_tensor("hT", [hid, npc], F32, kind="ExternalOutput").ap()

    with tile.TileContext(nc) as tc, ExitStack() as ctx:
        consts = ctx.enter_context(tc.tile_pool(name="consts", bufs=1))
        macro_pool = ctx.enter_context(tc.tile_pool(name="macro", bufs=2))
        edge_pool = ctx.enter_context(tc.tile_pool(name="edge", bufs=2))
        work = ctx.enter_context(tc.tile_pool(name="work", bufs=2))
        psum = ctx.enter_context(tc.tile_pool(name="psum", bufs=2, space="PSUM"))

        # ---- weights (resident) ----
        # wc_sb[k] holds Wc rows [128k:128k+128]; k in [0, kx] are x chunks,
        # k == kx is the h_tilde chunk. Columns: 3*hid (z_i | z_o | z_u).
        wc_sb = []
        for k in range(kx + 1):
            t = consts.tile([128, 3 * hid], F32R, tag=f"wc{k}")
            nc.sync.dma_start(out=t, in_=Wc[128 * k : 128 * (k + 1), :])
            wc_sb.append(t)
        wf_sb = []
        for k in range(kx):
            t = consts.tile([128, hid], F32R, tag=f"wf{k}")
            nc.sync.dma_start(out=t, in_=Wf[128 * k : 128 * (k + 1), :])
            wf_sb.append(t)
        uf_sb = consts.tile([128, hid], F32R, tag="uf")
        nc.sync.dma_start(out=uf_sb, in_=Uf)
        bc_sb = consts.tile([128, 3], F32, tag="bc")
        nc.sync.dma_start(out=bc_sb, in_=bc)
        bf_sb = consts.tile([128, 1], F32, tag="bf")
        nc.sync.dma_start(out=bf_sb, in_=bf)

        ACTF = mybir.ActivationFunctionType
        fjc_eng = getattr(nc, eng["fjc"])
        fc_eng = getattr(nc, eng["fc"])
        gate_eng = getattr(nc, eng["gates"])

        for m0, msz in _chunks(npc, MACRO):
            x_sb = []
            for k in range(kx):
                t = macro_pool.tile([128, msz], F32R, tag=f"x{k}")
                nc.sync.dma_start(out=t, in_=xT[128 * k : 128 * (k + 1), m0 : m0 + msz])
                x_sb.append(t)
            c_out = macro_pool.tile([128, msz], F32, tag="c_out")
            h_out = macro_pool.tile([128, msz], F32, tag="h_out")

            for s0, ssz in _chunks(msz, SUB):
                n0 = m0 + s0  # absolute node offset
                esz = ssz * cpn

                ch_sb = edge_pool.tile([128, esz], F32R, tag="ch")
                nc.sync.dma_start(out=ch_sb, in_=chT[:, n0 * cpn : n0 * cpn + esz])
                cc_sb = edge_pool.tile([128, esz], F32, tag="cc")
                nc.sync.dma_start(out=cc_sb, in_=ccT[:, n0 * cpn : n0 * cpn + esz])

                ch3 = ch_sb.rearrange("p (n c) -> p n c", c=cpn)

                # ---- z = big_in @ W_combined + b_combined (transposed) ----
                zp = []
                for j in range(3):  # z_i, z_o, z_u output chunks
                    p = psum.tile([128, ssz], F32, tag=f"z{j}")
                    for k in range(kx):
                        nc.tensor.matmul(
                            p,
                            lhsT=wc_sb[k][:, 128 * j : 128 * (j + 1)],
                            rhs=x_sb[k][:, s0 : s0 + ssz],
                            start=(k == 0),
                            stop=False,
                        )
                    for c in range(cpn):  # += W_h^T @ h_tilde^T, child-sum folded in
                        nc.tensor.matmul(
                            p,
                            lhsT=wc_sb[kx][:, 128 * j : 128 * (j + 1)],
                            rhs=ch3[:, :, c],
                            start=False,
                            stop=(c == cpn - 1),
                        )
                    zp.append(p)

                sig_i = work.tile([128, ssz], F32, tag="sig_i")
                nc.scalar.activation(sig_i, zp[0], ACTF.Sigmoid, bias=bc_sb[:, 0:1])
                sig_o = work.tile([128, ssz], F32, tag="sig_o")
                nc.scalar.activation(sig_o, zp[1], ACTF.Sigmoid, bias=bc_sb[:, 1:2])
                tanh_u = work.tile([128, ssz], F32, tag="tanh_u")
                nc.scalar.activation(tanh_u, zp[2], ACTF.Tanh, bias=bc_sb[:, 2:3])

                # ---- f_jk = sigmoid(W_f x_j + U_f h_k + b_f), child-major ----
                # One PSUM tile per child slot c: f_c = U_f^T ch[:, c::cpn]
                # + W_f^T x (x term identical for all slots, plain slices —
                # no broadcast APs, which the fp32r ISA checker rejects).
                cc3 = cc_sb.rearrange("p (n c) -> p n c", c=cpn)
                fjc_c = []
                for c in range(cpn):
                    fp = psum.tile([128, ssz], F32, tag="fp")
                    nc.tensor.matmul(
                        fp, lhsT=uf_sb, rhs=ch3[:, :, c], start=True, stop=False
                    )
                    for k in range(kx):
                        nc.tensor.matmul(
                            fp,
                            lhsT=wf_sb[k],
                            rhs=x_sb[k][:, s0 : s0 + ssz],
                            start=False,
                            stop=(k == kx - 1),
                        )
                    f_c = work.tile([128, ssz], F32, tag=f"f{c}")
                    nc.scalar.activation(f_c, fp, ACTF.Sigmoid, bias=bf_sb[:, 0:1])
                    fjc = work.tile([128, ssz], F32, tag=f"fjc{c}")
                    fjc_eng.tensor_mul(fjc, f_c, cc3[:, :, c])
                    fjc_c.append(fjc)

                # ---- fc = segment_sum(f * child_c) ----
                if cpn == 1:
                    fc = fjc_c[0]
                else:
                    fc = work.tile([128, ssz], F32, tag="fc")
                    fc_eng.tensor_add(fc, fjc_c[0], fjc_c[1])
                    for ci in range(2, cpn):
                        fc_eng.tensor_add(fc, fc, fjc_c[ci])

                # ---- c, h ----
                c_sl = c_out[:, s0 : s0 + ssz]
                h_sl = h_out[:, s0 : s0 + ssz]
                gate_eng.tensor_mul(c_sl, sig_i, tanh_u)
                gate_eng.tensor_add(c_sl, c_sl, fc)
                tanh_c = work.tile([128, ssz], F32, tag="tanh_c")
                nc.scalar.activation(tanh_c, c_sl, ACTF.Tanh)
                gate_eng.tensor_mul(h_sl, sig_o, tanh_c)

            nc.sync.dma_start(out=cT[:, m0 : m0 + msz], in_=c_out)
            nc.sync.dma_start(out=hT[:, m0 : m0 + msz], in_=h_out)

    nc.compile()
    return nc


TRACE = False  # set True (e.g. from test.py) to capture an NTFF profile
LAST_RESULTS = None  # BassKernelResults of the most recent kernel() call

_PROGRAM_CACHE = {}


def _get_program(npc, in_dim, hid, cpn):
    key = (npc, in_dim, hid, cpn)
    if key not in _PROGRAM_CACHE:
        _PROGRAM_CACHE[key] = build_program(npc, in_dim, hid, cpn)
    return _PROGRAM_CACHE[key]


def _pad_children(child_c, child_h, segment_ids, n):
    """Regularize to exactly max_c children per node (zero padding is exact:
    padded slots contribute sigmoid(..)*0 to fc and 0 to the child sum)."""
    seg = np.asarray(segment_ids).astype(np.int64)
    e = seg.shape[0]
    counts = np.bincount(seg, minlength=n)
    max_c = int(counts.max()) if e else 1
    if e == n * max_c and np.all(counts == max_c):
        return child_c, child_h, max_c  # already regular (and sorted)
    hid = child_h.shape[1]
    slot = np.arange(e, dtype=np.int64) - np.repeat(
        np.concatenate([[0], np.cumsum(counts)[:-1]]), counts
    )
    cc = np.zeros((n * max_c, hid), np.float32)
    ch = np.zeros((n * max_c, hid), np.float32)
    idx = seg * max_c + slot
    cc[idx] = child_c
    ch[idx] = child_h
    return cc, ch, max_c


def kernel(
    inputs,
    child_c,
    child_h,
    segment_ids,
    W_combined,
    b_combined,
    W_f,
    U_f,
    b_f,
):
    inputs = np.asarray(inputs, dtype=np.float32)
    child_c = np.asarray(child_c, dtype=np.float32)
    child_h = np.asarray(child_h, dtype=np.float32)
    n, in_dim = inputs.shape
    hid = U_f.shape[0]

    child_c, child_h, cpn = _pad_children(child_c, child_h, segment_ids, n)

    assert n % N_CORES == 0
    npc = n // N_CORES

    nc = _get_program(npc, in_dim, hid, cpn)

    Wc = np.ascontiguousarray(np.asarray(W_combined, dtype=np.float32))
    Wf = np.ascontiguousarray(np.asarray(W_f, dtype=np.float32))
    Uf = np.ascontiguousarray(np.asarray(U_f, dtype=np.float32))
    bc = np.ascontiguousarray(
        np.asarray(b_combined, dtype=np.float32).reshape(3, hid).T
    )
    bf = np.ascontiguousarray(np.asarray(b_f, dtype=np.float32).reshape(hid, 1))

    in_maps = []
    for c in range(N_CORES):
        n0, n1 = c * npc, (c + 1) * npc
        e0, e1 = n0 * cpn, n1 * cpn
        in_maps.append(
            {
                "xT": np.ascontiguousarray(inputs[n0:n1].T),
                "chT": np.ascontiguousarray(child_h[e0:e1].T),
                "ccT": np.ascontiguousarray(child_c[e0:e1].T),
                "Wc": Wc,
                "Wf": Wf,
                "Uf": Uf,
                "bc": bc,
                "bf": bf,
            }
        )

    res = run_bass_kernel_spmd(
        nc, in_maps, core_ids=list(range(N_CORES)), trace=TRACE
    )
    global LAST_RESULTS
    LAST_RESULTS = res

    c_full = np.empty((n, hid), np.float32)
    h_full = np.empty((n, hid), np.float32)
    for c in range(N_CORES):
        n0, n1 = c * npc, (c + 1) * npc
        c_full[n0:n1] = res.results[c]["cT"].T
        h_full[n0:n1] = res.results[c]["hT"].T
    return (c_full, h_full)


if __name__ == "__main__":
    # tiny smoke test against a numpy reference
    rng = np.random.default_rng(0)
    n, in_dim, hid, cpn = 2 * N_CORES * MACRO // 25, 256, 128, 3  # small-ish
    print(f"smoke: n={n}")


# revision 13
# speedup vs baseline: 1.4236x; 1.4236x over previous
"""Batch Child-Sum TreeLSTM cell on 8 Trainium2 NeuronCores.

Strategy (data-parallel over nodes):
  - Shard the N nodes (and their contiguous 3-child edge segments) evenly
    across the 8 cores; replicate the small weight matrices.
  - On the host, lay activations out feature-major ("transposed", features on
    SBUF partitions) so every matmul contraction is along partitions and all
    segment sums are cheap strided free-axis ops.
  - All matmuls run as float32r (full PE rate at moving free-dim >= 256).
  - The child-sum (h_tilde) is folded into the big matmul as three extra
    accumulating matmuls over stride-3 views of child_h^T.
  - The per-edge forget-gate preactivation W_f x_j + U_f h_k is accumulated in
    a single PSUM tile (broadcast moving AP for the x term), and ScalarE
    evacuates it as sigmoid(psum + b_f) in one op.
"""

from contextlib import ExitStack

import numpy as np

import concourse.bass as bass
import concourse.bacc as bacc
import concourse.tile as tile
from concourse import mybir
from concourse.bass_utils import run_bass_kernel_spmd

F32 = mybir.dt.float32
F32R = mybir.dt.float32r
BF16 = mybir.dt.bfloat16

# Matmul operand dtype. "bf16": x/child_h are cast to bf16 during the load
# DMA (SWDGE) and weights are passed as bf16 -> single-pass matmuls at full
# PE rate. "fp32r": everything stays 32-bit (2-pass matmuls, ~2x PE time)
# but ~bf16x2 precision.
MM_DT = "bf16"

N_CORES = 8

# Tiling (in nodes). MACRO: DMA granularity for x / outputs. SUB: matmul
# moving free dim / PSUM tile width. FGRP: node group per forget-gate PSUM
# tile (FGRP * cpn must be <= 512 to fit one PSUM bank / one fp32 matmul,
# and even — walrus's fp32r dst-mem-pattern check rejects odd free dims).
MACRO = 2500
SUB = 500
FGRP = 170


def _ceil_div(a, b):
    return (a + b - 1) // b


def _chunks(total, step):
    out = []
    off = 0
    while off < total:
        out.append((off, min(step, total - off)))
        off += step
    return out


def build_program(npc, in_dim, hid, cpn, engines=None):
    """Bass program for one core's shard: npc nodes, npc*cpn edges."""
    assert in_dim % 128 == 0 and hid == 128
    kx = in_dim // 128  # k-chunks of the input features
    epc = npc * cpn

    # gpsimd is poison for fine-grained work here: its EVENT_SEMAPHORE ops are
    # ucode-dispatched (~1.3us each) and dominate its engine span.
    eng = {"fjc": "vector", "fc": "vector", "gates": "vector"}
    if engines:
        eng.update(engines)

    mm_bf16 = MM_DT == "bf16"
    MMDT = BF16 if mm_bf16 else F32R
    WDT = BF16 if mm_bf16 else F32R

    nc = bacc.Bacc("TRN2", target_bir_lowering=False, debug=False)

    xT = nc.dram_tensor("xT", [in_dim, npc], F32 if mm_bf16 else F32R, kind="ExternalInput").ap()
    chT = nc.dram_tensor("chT", [hid, epc], F32 if mm_bf16 else F32R, kind="ExternalInput").ap()
    ccT = nc.dram_tensor("ccT", [hid, epc], F32, kind="ExternalInput").ap()
    # W_combined [in+hid, 3*hid] row-major, sliced into [128,128] chunks.
    Wc = nc.dram_tensor("Wc", [in_dim + hid, 3 * hid], WDT, kind="ExternalInput").ap()
    Wf = nc.dram_tensor("Wf", [in_dim, hid], WDT, kind="ExternalInput").ap()
    Uf = nc.dram_tensor("Uf", [hid, hid], WDT, kind="ExternalInput").ap()
    bc = nc.dram_tensor("bc", [hid, 3], F32, kind="ExternalInput").ap()  # b_combined.reshape(3,128).T
    bf = nc.dram_tensor("bf", [hid, 1], F32, kind="ExternalInput").ap()

    cT = nc.dram_tensor("cT", [hid, npc], F32, kind="ExternalOutput").ap()
    hT = nc.dram_tensor("hT", [hid, npc], F32, kind="ExternalOutput").ap()

    with tile.TileContext(nc) as tc, ExitStack() as ctx:
        consts = ctx.enter_context(tc.tile_pool(name="consts", bufs=1))
        macro_pool = ctx.enter_context(tc.tile_pool(name="macro", bufs=2))
        edge_pool = ctx.enter_context(tc.tile_pool(name="edge", bufs=3))
        work = ctx.enter_context(tc.tile_pool(name="work", bufs=2))
        psum = ctx.enter_context(tc.tile_pool(name="psum", bufs=2, space="PSUM"))

        # ---- weights (resident) ----
        # wc_sb[k] holds Wc rows [128k:128k+128]; k in [0, kx] are x chunks,
        # k == kx is the h_tilde chunk. Columns: 3*hid (z_i | z_o | z_u).
        wc_sb = []
        for k in range(kx + 1):
            t = consts.tile([128, 3 * hid], WDT, tag=f"wc{k}")
            nc.sync.dma_start(out=t, in_=Wc[128 * k : 128 * (k + 1), :])
            wc_sb.append(t)
        wf_sb = []
        for k in range(kx):
            t = consts.tile([128, hid], WDT, tag=f"wf{k}")
            nc.sync.dma_start(out=t, in_=Wf[128 * k : 128 * (k + 1), :])
            wf_sb.append(t)
        uf_sb = consts.tile([128, hid], WDT, tag="uf")
        nc.sync.dma_start(out=uf_sb, in_=Uf)
        bc_sb = consts.tile([128, 3], F32, tag="bc")
        nc.sync.dma_start(out=bc_sb, in_=bc)
        bf_sb = consts.tile([128, 1], F32, tag="bf")
        nc.sync.dma_start(out=bf_sb, in_=bf)

        ACTF = mybir.ActivationFunctionType
        fjc_eng = getattr(nc, eng["fjc"])
        fc_eng = getattr(nc, eng["fc"])
        gate_eng = getattr(nc, eng["gates"])

        for m0, msz in _chunks(npc, MACRO):
            x_sb = []
            for k in range(kx):
                t = macro_pool.tile([128, msz], MMDT, tag=f"x{k}")
                x_dma = nc.gpsimd if mm_bf16 else nc.sync  # SWDGE casts f32->bf16
                x_dma.dma_start(out=t, in_=xT[128 * k : 128 * (k + 1), m0 : m0 + msz])
                x_sb.append(t)
            c_out = macro_pool.tile([128, msz], F32, tag="c_out")
            h_out = macro_pool.tile([128, msz], F32, tag="h_out")

            for s0, ssz in _chunks(msz, SUB):
                n0 = m0 + s0  # absolute node offset
                esz = ssz * cpn

                ch_sb = edge_pool.tile([128, esz], MMDT, tag="ch")
                (nc.gpsimd if mm_bf16 else nc.sync).dma_start(
                    out=ch_sb, in_=chT[:, n0 * cpn : n0 * cpn + esz]
                )
                cc_sb = edge_pool.tile([128, esz], F32, tag="cc")
                nc.sync.dma_start(out=cc_sb, in_=ccT[:, n0 * cpn : n0 * cpn + esz])

                ch3 = ch_sb.rearrange("p (n c) -> p n c", c=cpn)

                # ---- z = big_in @ W_combined + b_combined (transposed) ----
                zp = []
                for j in range(3):  # z_i, z_o, z_u output chunks
                    p = psum.tile([128, ssz], F32, tag=f"z{j}")
                    for k in range(kx):
                        nc.tensor.matmul(
                            p,
                            lhsT=wc_sb[k][:, 128 * j : 128 * (j + 1)],
                            rhs=x_sb[k][:, s0 : s0 + ssz],
                            start=(k == 0),
                            stop=False,
                        )
                    for c in range(cpn):  # += W_h^T @ h_tilde^T, child-sum folded in
                        nc.tensor.matmul(
                            p,
                            lhsT=wc_sb[kx][:, 128 * j : 128 * (j + 1)],
                            rhs=ch3[:, :, c],
                            start=False,
                            stop=(c == cpn - 1),
                        )
                    zp.append(p)

                sig_i = work.tile([128, ssz], F32, tag="sig_i")
                nc.scalar.activation(sig_i, zp[0], ACTF.Sigmoid, bias=bc_sb[:, 0:1])
                sig_o = work.tile([128, ssz], F32, tag="sig_o")
                nc.scalar.activation(sig_o, zp[1], ACTF.Sigmoid, bias=bc_sb[:, 1:2])
                tanh_u = work.tile([128, ssz], F32, tag="tanh_u")
                nc.scalar.activation(tanh_u, zp[2], ACTF.Tanh, bias=bc_sb[:, 2:3])

                # ---- f_jk = sigmoid(W_f x_j + U_f h_k + b_f), child-major ----
                # One PSUM tile per child slot c: f_c = U_f^T ch[:, c::cpn]
                # + W_f^T x (x term identical for all slots, plain slices —
                # no broadcast APs, which the fp32r ISA checker rejects).
                cc3 = cc_sb.rearrange("p (n c) -> p n c", c=cpn)
                fjc_c = []
                for c in range(cpn):
                    fp = psum.tile([128, ssz], F32, tag="fp")
                    nc.tensor.matmul(
                        fp, lhsT=uf_sb, rhs=ch3[:, :, c], start=True, stop=False
                    )
                    for k in range(kx):
                        nc.tensor.matmul(
                            fp,
                            lhsT=wf_sb[k],
                            rhs=x_sb[k][:, s0 : s0 + ssz],
                            start=False,
                            stop=(k == kx - 1),
                        )
                    f_c = work.tile([128, ssz], F32, tag=f"f{c}")
                    nc.scalar.activation(f_c, fp, ACTF.Sigmoid, bias=bf_sb[:, 0:1])
                    fjc = work.tile([128, ssz], F32, tag=f"fjc{c}")
                    fjc_eng.tensor_mul(fjc, f_c, cc3[:, :, c])
                    fjc_c.append(fjc)

                # ---- fc = segment_sum(f * child_c) ----
                if cpn == 1:
                    fc = fjc_c[0]
                else:
                    fc = work.tile([128, ssz], F32, tag="fc")
                    fc_eng.tensor_add(fc, fjc_c[0], fjc_c[1])
                    for ci in range(2, cpn):
                        fc_eng.tensor_add(fc, fc, fjc_c[ci])

                # ---- c, h ----
                c_sl = c_out[:, s0 : s0 + ssz]
                h_sl = h_out[:, s0 : s0 + ssz]
                gate_eng.tensor_mul(c_sl, sig_i, tanh_u)
                gate_eng.tensor_add(c_sl, c_sl, fc)
                tanh_c = work.tile([128, ssz], F32, tag="tanh_c")
                nc.scalar.activation(tanh_c, c_sl, ACTF.Tanh)
                gate_eng.tensor_mul(h_sl, sig_o, tanh_c)

            nc.sync.dma_start(out=cT[:, m0 : m0 + msz], in_=c_out)
            nc.sync.dma_start(out=hT[:, m0 : m0 + msz], in_=h_out)

    nc.compile()
    return nc


TRACE = False  # set True (e.g. from test.py) to capture an NTFF profile
LAST_RESULTS = None  # BassKernelResults of the most recent kernel() call

_PROGRAM_CACHE = {}


def _get_program(npc, in_dim, hid, cpn):
    key = (npc, in_dim, hid, cpn, MM_DT, MACRO, SUB)
    if key not in _PROGRAM_CACHE:
        _PROGRAM_CACHE[key] = build_program(npc, in_dim, hid, cpn)
    return _PROGRAM_CACHE[key]


def _pad_children(child_c, child_h, segment_ids, n):
    """Regularize to exactly max_c children per node (zero padding is exact:
    padded slots contribute sigmoid(..)*0 to fc and 0 to the child sum)."""
    seg = np.asarray(segment_ids).astype(np.int64)
    e = seg.shape[0]
    counts = np.bincount(seg, minlength=n)
    max_c = int(counts.max()) if e else 1
    if e == n * max_c and np.all(counts == max_c):
        return child_c, child_h, max_c  # already regular (and sorted)
    hid = child_h.shape[1]
    slot = np.arange(e, dtype=np.int64) - np.repeat(
        np.concatenate([[0], np.cumsum(counts)[:-1]]), counts
    )
    cc = np.zeros((n * max_c, hid), np.float32)
    ch = np.zeros((n * max_c, hid), np.float32)
    idx = seg * max_c + slot
    cc[idx] = child_c
    ch[idx] = child_h
    return cc, ch, max_c


def kernel(
    inputs,
    child_c,
    child_h,
    segment_ids,
    W_combined,
    b_combined,
    W_f,
    U_f,
    b_f,
):
    inputs = np.asarray(inputs, dtype=np.float32)
    child_c = np.asarray(child_c, dtype=np.float32)
    child_h = np.asarray(child_h, dtype=np.float32)
    n, in_dim = inputs.shape
    hid = U_f.shape[0]

    child_c, child_h, cpn = _pad_children(child_c, child_h, segment_ids, n)

    assert n % N_CORES == 0
    npc = n // N_CORES

    nc = _get_program(npc, in_dim, hid, cpn)

    import ml_dtypes

    wdt = ml_dtypes.bfloat16 if MM_DT == "bf16" else np.float32
    Wc = np.ascontiguousarray(np.asarray(W_combined, dtype=np.float32).astype(wdt))
    Wf = np.ascontiguousarray(np.asarray(W_f, dtype=np.float32).astype(wdt))
    Uf = np.ascontiguousarray(np.asarray(U_f, dtype=np.float32).astype(wdt))
    bc = np.ascontiguousarray(
        np.asarray(b_combined, dtype=np.float32).reshape(3, hid).T
    )
    bf = np.ascontiguousarray(np.asarray(b_f, dtype=np.float32).reshape(hid, 1))

    in_maps = []
    for c in range(N_CORES):
        n0, n1 = c * npc, (c + 1) * npc
        e0, e1 = n0 * cpn, n1 * cpn
        in_maps.append(
            {
                "xT": np.ascontiguousarray(inputs[n0:n1].T),
                "chT": np.ascontiguousarray(child_h[e0:e1].T),
                "ccT": np.ascontiguousarray(child_c[e0:e1].T),
                "Wc": Wc,
                "Wf": Wf,
                "Uf": Uf,
                "bc": bc,
                "bf": bf,
            }
        )

    res = run_bass_kernel_spmd(
        nc, in_maps, core_ids=list(range(N_CORES)), trace=TRACE
    )
    global LAST_RESULTS
    LAST_RESULTS = res

    c_full = np.empty((n, hid), np.float32)
    h_full = np.empty((n, hid), np.float32)
    for c in range(N_CORES):
        n0, n1 = c * npc, (c + 1) * npc
        c_full[n0:n1] = res.results[c]["cT"].T
        h_full[n0:n1] = res.results[c]["hT"].T
    return (c_full, h_full)


if __name__ == "__main__":
    # tiny smoke test against a numpy reference
    rng = np.random.default_rng(0)
    n, in_dim, hid, cpn = 2 * N_CORES * MACRO // 25, 256, 128, 3  # small-ish
    print(f"smoke: n={n}")
